# revision 42
# baseline (speedup 1.0000x reference)
"""Trainium2 Bass kernel for causal self-attention (muP scaling).

Full-input contract: kernel(**inputs) takes the complete tensors and returns
the complete [B, T, C] output. Internally the work is split over 8 NeuronCores
as (batch b = core//2) x (head-group g = core%2, 8 heads each):

  - each core computes q,k,v for its batch restricted to its 8 heads,
    runs causal attention for those heads, and multiplies by the matching
    512-row slice of w_proj, producing a partial [T, C] output.
  - the host sums the two partials per batch and adds b_proj. No on-device
    collectives are needed.

v2: single software-pipelined stream. The QKV projection is chunked per
query-block qb: chunk(qb) computes exactly the new k columns (tb=qb), the
q columns for qb, and v tiles 4qb..4qb+3 - the data attention block qb
needs - so the first exp issues ~25us into the kernel and the ScalarE exp
stream (the second-largest engine load, ~155us) overlaps the remaining
projection matmuls instead of waiting for a serial phase 1.

Attention runs per head PAIR (even head at qkT partitions 0:64, odd at
64:128); the two K=64 score matmuls write the two halves of one
[128, 1024] PSUM tile which a single ScalarE exp drains (2-segment
strided AP, muP 1/64 scale folded in; no max-subtraction - logits are
~N(0, 0.13)). Causal masking is a 0/1 triangular multiply on
diagonal-crossing tiles only. attT-out[d, tq] accumulates v_aug.T @ expT
with an appended ones column, so row 64 of the accumulator is the softmax
denominator for free. AV matmuls of block b are paced between the score
matmuls of block b+1 (and the projection chunk at qb boundaries), so the
PE never stalls on ScalarE.

Normalization v2: right after a block's AV matmuls the accumulator is
copied out unnormalized (bf16) and its denominator row appended to a
per-qb [8, 512] staging tile, freeing the PSUM bank immediately. One
reciprocal_approx_fast per qb (custom DVE op, ~5x faster than the 6.5
cyc/elem iterative InstReciprocal, batched over all 8 head-halves)
produces the scales, which GpSimd partition-broadcasts and one bf16
multiply applies in place. This removes the 106us of DVE InstReciprocal
the v1 kernel spent normalizing per block-half.

The output projection for qb is emitted interleaved with block qb+1's
attention; y rides DMA from SBUF after a DVE PSUM->SBUF cast-copy.
Activations ride bf16; measured end-to-end error vs the fp32 reference
is ~4e-3 relative.
"""

import sys

if "/opt/trn_rl_repo" not in sys.path:
    sys.path.insert(0, "/opt/trn_rl_repo")

import numpy as np
import ml_dtypes

import concourse.bass as bass
import concourse.mybir as mybir
import concourse.tile as tile
from concourse import bacc
from concourse.bass_utils import run_bass_kernel_spmd
from concourse.masks import make_upper_triangular

# Problem shape (hardcoded per contract).
B, T, C, H = 4, 2048, 1024, 16
HD = C // H            # 64
N_CORES = 8
HG = H // 2            # 8 heads per core
GC = HG * HD           # 512 columns of q/k/v per core
P = 128                # SBUF partitions
CT = C // P            # 8 contraction tiles over C
TT = T // P            # 16 time tiles of 128
QB = 4                 # tq blocks
QW = T // QB           # 512 wide
KT = T // P            # 16 tk tiles

_bf16np = ml_dtypes.bfloat16
F32 = mybir.dt.float32
BF16 = mybir.dt.bfloat16
FP8 = mybir.dt.float8e4
_fp8np = mybir.dt.np(FP8)

_COMPILED = None


def _build_nc(reps=1):
    import os
    opts = {
        "pipeline": os.environ.get("KV2_PIPELINE", "1") == "1",
        "fast_recip": os.environ.get("KV2_FASTRECIP", "1") == "1",
        "chunk_p1": os.environ.get("KV2_CHUNK_P1", "1") == "1",
        "debug_den": os.environ.get("KV2_DEBUG_DEN", "0") == "1",
    }
    nc = bacc.Bacc("TRN2", target_bir_lowering=False, debug=False,
                   num_devices=N_CORES)

    xT = nc.dram_tensor("xT", [C, T], BF16, kind="ExternalInput").ap()
    # fp8 copies of x / w_qk in DoubleRow-paired layout: row r = ctp*128+p,
    # free (i, t): value for contraction dim c = ctp*256 + i*128 + p.
    x8 = nc.dram_tensor("x8", [C // 2, 2 * T], FP8, kind="ExternalInput").ap()
    w8 = nc.dram_tensor("w8", [C // 2, 4 * GC], FP8, kind="ExternalInput").ap()
    w_v = nc.dram_tensor("w_v", [C, GC], BF16, kind="ExternalInput").ap()
    b_qk = nc.dram_tensor("b_qk", [2 * GC], F32, kind="ExternalInput").ap()
    b_v = nc.dram_tensor("b_v", [GC], F32, kind="ExternalInput").ap()
    w_pr = nc.dram_tensor("w_pr", [GC, C], BF16, kind="ExternalInput").ap()
    y = nc.dram_tensor("y", [T, C], F32, kind="ExternalOutput").ap()
    dden = (nc.dram_tensor("dden", [4 * HG, QW], F32, kind="ExternalOutput").ap()
            if opts["debug_den"] else None)
    opts = dict(opts, dden=dden)
    del opts["debug_den"]

    with tile.TileContext(nc) as tc:
        for _ in range(reps):
            _emit(nc, tc, xT, x8, w8, w_v, b_qk, b_v, w_pr, y, **opts)
    nc.finalize()
    return nc


def _emit(nc, tc, xT, x8, w8, w_v, b_qk, b_v, w_pr, y,
          pipeline=True, fast_recip=True, chunk_p1=True, dden=None):
    from contextlib import ExitStack

    ctx = ExitStack()
    with ctx:
        persist = ctx.enter_context(tc.tile_pool(name="persist", bufs=1))

        # ---- constants -------------------------------------------------
        tri = persist.tile([P, P], BF16, tag="tri")     # 0/1, 1 iff j >= i
        make_upper_triangular(nc, tri[:, :], val=1.0, diag=True)

        bqk_sb = persist.tile([P, CT], F32, tag="bqk")  # [128, 8] col jt
        nc.sync.dma_start(
            out=bqk_sb[:, :],
            in_=bass.AP(tensor=b_qk.tensor, offset=0, ap=[[1, P], [P, CT]]),
        )
        bv_sb = persist.tile([P, GC], F32, tag="bv")
        nc.gpsimd.dma_start(
            out=bv_sb[:, :],
            in_=bass.AP(tensor=b_v.tensor, offset=0, ap=[[0, P], [1, GC]]),
        )

        # ---- persistent tiles ------------------------------------------
        xts = [persist.tile([P, T], BF16, name=f"xT{ct}", tag=f"xT{ct}")
               for ct in range(CT)]
        x8t = [persist.tile([P, 2, T], FP8, name=f"x8_{cp}", tag=f"x8_{cp}")
               for cp in range(CT // 2)]
        w8t = [persist.tile([P, 2, 2 * GC], FP8, name=f"w8_{cp}", tag=f"w8_{cp}")
               for cp in range(CT // 2)]
        wvts = [persist.tile([P, GC], BF16, name=f"wv{ct}", tag=f"wv{ct}")
                for ct in range(CT)]
        qkT = [persist.tile([P, T], BF16, name=f"qkT{j}", tag=f"qkT{j}")
               for j in range(CT)]
        v_sb = [persist.tile([P, HG, HD + 1], BF16, name=f"v{t}", tag=f"v{t}")
                for t in range(TT)]
        att = [persist.tile([P, T], BF16, name=f"att{j}", tag=f"att{j}")
               for j in range(CT // 2)]
        wpr = [persist.tile([P, C], BF16, name=f"wpr{j}", tag=f"wpr{j}")
               for j in range(CT // 2)]

        # ---- input DMAs, deadline order, spread over the 3 hwdge queues
        _q = [nc.sync, nc.scalar, nc.gpsimd]
        qi = [0]

        def dma_in(out, in_):
            _q[qi[0] % 3].dma_start(out=out, in_=in_)
            qi[0] += 1

        # x8(tb=0) and the k half of w8 feed the very first matmuls.
        x8v = [x8[cp * P:(cp + 1) * P, :].rearrange("p (i t) -> p i t", i=2)
               for cp in range(CT // 2)]
        w8v = [w8[cp * P:(cp + 1) * P, :].rearrange("p (i j) -> p i j", i=2)
               for cp in range(CT // 2)]
        for cp in range(CT // 2):
            dma_in(x8t[cp][:, :, 0:QW], x8v[cp][:, :, 0:QW])
            dma_in(w8t[cp][:, :, GC:2 * GC], w8v[cp][:, :, GC:2 * GC])
        for cp in range(CT // 2):
            dma_in(w8t[cp][:, :, 0:GC], w8v[cp][:, :, 0:GC])
        for ct in range(CT):   # bf16 x cols for the first v tiles
            dma_in(xts[ct][:, 0:QW], xT[ct * P:(ct + 1) * P, 0:QW])
        for ct in range(CT):
            dma_in(wvts[ct][:, :], w_v[ct * P:(ct + 1) * P, :])
        for tb in range(1, QB):
            for cp in range(CT // 2):
                dma_in(x8t[cp][:, :, tb * QW:(tb + 1) * QW],
                       x8v[cp][:, :, tb * QW:(tb + 1) * QW])
            for ct in range(CT):
                dma_in(xts[ct][:, tb * QW:(tb + 1) * QW],
                       xT[ct * P:(ct + 1) * P, tb * QW:(tb + 1) * QW])
        for ct in range(CT // 2):
            dma_in(wpr[ct][:, :], w_pr[ct * P:(ct + 1) * P, :])

        # ---- pools -----------------------------------------------------
        expp = ctx.enter_context(tc.tile_pool(name="expp", bufs=20))
        nrm = ctx.enter_context(tc.tile_pool(name="nrm", bufs=2))
        yp = ctx.enter_context(tc.tile_pool(name="ysb", bufs=2))
        pss = ctx.enter_context(tc.tile_pool(name="pss", bufs=2, space="PSUM"))
        # role-split PSUM rings: block accumulators (long-lived, 2 slots is
        # exactly one block in flight) vs transient p1/proj tiles. Keeping
        # them separate preserves the ring-order invariants no matter how
        # many side jobs are interleaved between attention blocks.
        pacc = ctx.enter_context(tc.tile_pool(name="pacc", bufs=2, space="PSUM"))
        pfx = ctx.enter_context(tc.tile_pool(name="pfx", bufs=2, space="PSUM"))

        # ---- phase-1 chunk jobs (paired for PSUM bank alternation) -----
        def p1_kq(jts, tb):
            """k or q row tiles jts (absolute qkT index) for time block tb.

            fp8e4 DoubleRow: each matmul contracts 256 c-dims (2 planes of
            128) at half the per-column cost of bf16."""
            pss_ = [pfx.tile([P, QW], F32, name=f"p1_{jt}_{tb}", tag="fx")
                    for jt in jts]
            for cp in range(CT // 2):
                for i, jt in enumerate(jts):
                    nc.tensor.matmul(
                        pss_[i][:, :],
                        w8t[cp][:, :, jt * P:(jt + 1) * P],
                        x8t[cp][:, :, tb * QW:(tb + 1) * QW],
                        start=(cp == 0), stop=(cp == CT // 2 - 1),
                        perf_mode=mybir.MatmulPerfMode.DoubleRow,
                    )
            for i, jt in enumerate(jts):
                nc.vector.tensor_scalar_add(
                    out=qkT[jt][:, tb * QW:(tb + 1) * QW],
                    in0=pss_[i][:, :],
                    scalar1=bqk_sb[:, jt:jt + 1],
                )

        def p1_v(tts):
            pss_ = [pfx.tile([P, GC], F32, name=f"p1v_{tt}", tag="fx")
                    for tt in tts]
            for ct in range(CT):
                for i, tt in enumerate(tts):
                    nc.tensor.matmul(
                        pss_[i][:, :],
                        xts[ct][:, tt * P:(tt + 1) * P],
                        wvts[ct][:, :],
                        start=(ct == 0), stop=(ct == CT - 1),
                    )
            for i, tt in enumerate(tts):
                nc.vector.tensor_add(
                    out=v_sb[tt][:, :, 0:HD],
                    in0=pss_[i][:, :].rearrange("p (h e) -> p h e", e=HD),
                    in1=bv_sb[:, :].rearrange("p (h e) -> p h e", e=HD),
                )
                nc.vector.memset(v_sb[tt][:, :, HD:HD + 1], 1.0)

        # ---- attention emitters ---------------------------------------
        def emit_scores(hp, q0, kt, off, crossing):
            n = QW - off
            qT_t, kT_t = qkT[hp], qkT[CT // 2 + hp]
            ex = expp.tile([P, 2 * QW], BF16, tag="exp")
            ps = pss.tile([P, 2 * QW], F32, tag="sc")
            nc.tensor.matmul(
                ps[:, 0:n],
                kT_t[0:HD, kt * P:(kt + 1) * P],
                qT_t[0:HD, q0 + off:q0 + QW],
                start=True, stop=True,
            )
            nc.tensor.matmul(
                ps[:, QW:QW + n],
                kT_t[HD:P, kt * P:(kt + 1) * P],
                qT_t[HD:P, q0 + off:q0 + QW],
                start=True, stop=True,
            )
            # one exp over both heads: 2-segment strided view
            ps2 = ps[:, :].rearrange("p (s q) -> p s q", s=2)
            ex2 = ex[:, :].rearrange("p (s q) -> p s q", s=2)
            nc.scalar.activation(
                out=ex2[:, :, 0:n], in_=ps2[:, :, 0:n],
                func=mybir.ActivationFunctionType.Exp,
                scale=1.0 / HD,
            )
            if crossing:
                nc.vector.tensor_mul(
                    out=ex[:, 0:P], in0=ex[:, 0:P], in1=tri[:, :])
                nc.vector.tensor_mul(
                    out=ex[:, QW:QW + P], in0=ex[:, QW:QW + P], in1=tri[:, :])
            return ex

        def emit_av(st, i):
            (qb, hp, accs, exps, _prog) = st
            kt, off, n, ex = exps[i]
            last = i == len(exps) - 1
            nc.tensor.matmul(
                accs[0][0:HD + 1, off:QW],
                v_sb[kt][:, 2 * hp, :],
                ex[:, 0:n],
                start=(i == 0), stop=last,
                skip_group_check=True,
            )
            nc.tensor.matmul(
                accs[1][0:HD + 1, off:QW],
                v_sb[kt][:, 2 * hp + 1, :],
                ex[:, QW:QW + n],
                start=(i == 0), stop=last,
                skip_group_check=True,
            )

        def emit_post(st):
            """After a block's AVs: normalize straight out of PSUM.

            reciprocal_approx_fast is a single-pass custom DVE op (~0.6us on
            [1,512] vs 3.3us for the iterative InstReciprocal), so per-half
            normalization is cheap without any cross-partition batching."""
            (qb, hp, accs, exps, _prog) = st
            q0 = qb * QW
            for half, acc in ((0, accs[0]), (1, accs[1])):
                r0 = half * HD
                rec = nrm.tile([1, QW], F32, tag="rec", bufs=4)
                if fast_recip:
                    # custom DVE ops ignore the input AP's base partition
                    # (read physical row 0) - stage the denominator row at
                    # partition 0 with a plain copy (which does handle
                    # cross-base) before running the approx reciprocal.
                    den = nrm.tile([1, QW], F32, tag="den", bufs=2)
                    nc.vector.tensor_copy(out=den[0:1, :],
                                          in_=acc[HD:HD + 1, :])
                    if dden is not None:
                        nc.sync.dma_start(
                            out=dden[qb * CT + 2 * hp + half:
                                     qb * CT + 2 * hp + half + 1, :],
                            in_=den[0:1, :])
                    nc.vector.reciprocal_approx_fast(
                        out=rec[0:1, :], in_=den[0:1, :])
                else:
                    nc.vector.reciprocal(
                        out=rec[0:1, :], in_=acc[HD:HD + 1, :])
                bc = nrm.tile([HD, QW], F32, tag="bc", bufs=4)
                nc.gpsimd.partition_broadcast(
                    bc[0:HD, :], rec[0:1, :], channels=HD)
                nc.vector.tensor_mul(
                    out=att[hp][r0:r0 + HD, q0:q0 + QW],
                    in0=acc[0:HD, :],
                    in1=bc[0:HD, :],
                )

        yq = [0]

        def emit_proj_tile(tt):
            ysb = yp.tile([P, C], F32, tag="y")
            for nb in range(2):
                ps = pfx.tile([P, QW], F32, tag="fx", name=f"pj_{tt}_{nb}")
                for ct in range(CT // 2):
                    nc.tensor.matmul(
                        ps[:, :],
                        att[ct][:, tt * P:(tt + 1) * P],
                        wpr[ct][:, nb * QW:(nb + 1) * QW],
                        start=(ct == 0), stop=(ct == CT // 2 - 1),
                    )
                nc.vector.tensor_copy(
                    out=ysb[:, nb * QW:(nb + 1) * QW], in_=ps[:, :])
            (nc.sync if yq[0] % 2 == 0 else nc.gpsimd).dma_start(
                out=y[tt * P:(tt + 1) * P, :], in_=ysb[:, :])
            yq[0] += 1

        # ---- main pipelined loop --------------------------------------
        pend = [None]

        def pace_pend(frac_hi):
            st = pend[0]
            if st is None:
                return
            npend = len(st[3])
            hi = npend if frac_hi >= 1.0 else min(npend, int(frac_hi * npend))
            prog = st[4]
            for j in range(prog[0], hi):
                emit_av(st, j)
            prog[0] = max(prog[0], hi)
            if prog[0] >= npend:
                emit_post(st)
                pend[0] = None

        def p1_jobs_for(tb):
            return [
                lambda: p1_kq([4, 5], tb),
                lambda: p1_kq([0, 1], tb),
                lambda: p1_kq([6, 7], tb),
                lambda: p1_kq([2, 3], tb),
                lambda: p1_v([4 * tb, 4 * tb + 1]),
                lambda: p1_v([4 * tb + 2, 4 * tb + 3]),
            ]

        # prologue: data for the first query block
        if chunk_p1:
            for job in p1_jobs_for(0):
                job()
        else:
            for tb in range(QB):
                for job in p1_jobs_for(tb):
                    job()

        for qb in range(QB):
            # side work interleaved into this qb's block gaps: the NEXT qb's
            # projection chunks (so its scores can start without a bunched
            # serial phase-1 stall) and the PREVIOUS qb's output projection.
            side = []
            if chunk_p1 and qb + 1 < QB:
                side += p1_jobs_for(qb + 1)
            if qb > 0:
                side += [(lambda tt=tt: emit_proj_tile(tt))
                         for tt in range(4 * (qb - 1), 4 * qb)]

            for hp in range(HG // 2):
                tiles = [(kt, 0, False) for kt in range(4 * qb)]
                tiles += [(4 * qb + a, P * a, True) for a in range(4)]
                q0 = qb * QW
                acc_e = pacc.tile([P, QW], F32, name=f"acc_e{hp}_{qb}", tag="acc")
                acc_o = pacc.tile([P, QW], F32, name=f"acc_o{hp}_{qb}", tag="acc")
                exps = []
                nt = len(tiles)
                for i, (kt, off, crossing) in enumerate(tiles):
                    ex = emit_scores(hp, q0, kt, off, crossing)
                    exps.append((kt, off, QW - off, ex))
                    pace_pend((i + 1) / nt)
                pace_pend(1.0)
                pend[0] = (qb, hp, (acc_e, acc_o), exps, [0])
                if not pipeline:
                    pace_pend(1.0)  # drain immediately (no AV/scores overlap)
                # spread side jobs over the 4 block gaps (ceil-even split)
                gaps_left = HG // 2 - hp
                take = (len(side) + gaps_left - 1) // gaps_left
                for _ in range(take):
                    side.pop(0)()
            assert not side

        # tail: last block, last projections
        pace_pend(1.0)
        for tt in range(4 * (QB - 1), 4 * QB):
            emit_proj_tile(tt)


def _get_compiled():
    global _COMPILED
    if _COMPILED is None:
        _COMPILED = _build_nc()
    return _COMPILED


def _pair_fp8(a):
    """[C, n] -> DoubleRow-paired fp8 [C//2, 2n]: row r=cp*128+p holds
    (plane i, col j) = a[cp*256 + i*128 + p, j]."""
    n = a.shape[1]
    return np.ascontiguousarray(
        a.reshape(CT // 2, 2, P, n).transpose(0, 2, 1, 3).reshape(C // 2, 2 * n)
    ).astype(_fp8np)


def _make_in_maps(x, w_qkv, b_qkv, w_proj):
    in_maps = []
    for c in range(N_CORES):
        b, g = c // 2, c % 2
        s = slice(g * GC, (g + 1) * GC)
        xTb = np.ascontiguousarray(x[b].T)
        w_qk = np.concatenate(
            [w_qkv[:, s], w_qkv[:, C + g * GC:C + (g + 1) * GC]], axis=1)
        in_maps.append({
            "xT": xTb.astype(_bf16np),
            "x8": _pair_fp8(xTb),
            "w8": _pair_fp8(w_qk),
            "w_v": np.ascontiguousarray(
                w_qkv[:, 2 * C + g * GC:2 * C + (g + 1) * GC]).astype(_bf16np),
            "b_qk": np.ascontiguousarray(
                np.concatenate([b_qkv[s], b_qkv[C + g * GC:C + (g + 1) * GC]])),
            "b_v": np.ascontiguousarray(b_qkv[2 * C + g * GC:2 * C + (g + 1) * GC]),
            "w_pr": np.ascontiguousarray(w_proj[g * GC:(g + 1) * GC, :]).astype(_bf16np),
        })
    return in_maps


_RUNNER = None


def _get_runner():
    """Compile once, cache the jitted shard_map executable across calls."""
    global _RUNNER
    if _RUNNER is not None:
        return _RUNNER
    import jax
    from jax.sharding import Mesh, PartitionSpec, NamedSharding
    from jax.experimental.shard_map import shard_map
    from concourse.bass2jax import (_bass_exec_p, install_neuronx_cc_hook,
                                    partition_id_tensor)

    nc = _get_compiled()
    install_neuronx_cc_hook()
    partition_name = nc.partition_id_tensor.name if nc.partition_id_tensor else None
    in_names, out_names, out_avals, zero_outs = [], [], [], []
    for alloc in nc.m.functions[0].allocations:
        if not isinstance(alloc, mybir.MemoryLocationSet):
            continue
        name = alloc.memorylocations[0].name
        if alloc.kind == "ExternalInput":
            if name != partition_name:
                in_names.append(name)
        elif alloc.kind == "ExternalOutput":
            out_names.append(name)
            out_avals.append(jax.core.ShapedArray(tuple(alloc.tensor_shape),
                                                  mybir.dt.np(alloc.dtype)))
            zero_outs.append(np.zeros(tuple(alloc.tensor_shape),
                                      mybir.dt.np(alloc.dtype)))
    all_in = list(in_names) + list(out_names)
    if partition_name:
        all_in.append(partition_name)

    def _body(*args):
        ops = list(args)
        if partition_name:
            ops.append(partition_id_tensor())
        return tuple(_bass_exec_p.bind(
            *ops, out_avals=tuple(out_avals), in_names=tuple(all_in),
            out_names=tuple(out_names), lowering_input_output_aliases=(),
            sim_require_finite=True, sim_require_nnan=True, nc=nc))

    devices = jax.devices()[:N_CORES]
    mesh = Mesh(np.asarray(devices), ("core",))
    sharded = jax.jit(shard_map(
        _body, mesh=mesh,
        in_specs=(PartitionSpec("core"),) * (len(in_names) + len(out_avals)),
        out_specs=(PartitionSpec("core"),) * len(out_avals), check_rep=False),
        keep_unused=True)
    sharding = NamedSharding(mesh, PartitionSpec("core"))
    _RUNNER = (sharded, in_names, zero_outs, sharding, out_avals, out_names)
    return _RUNNER


def _execute(in_maps):
    import jax
    sharded, in_names, zero_outs, sharding, out_avals, out_names = _get_runner()
    ci = [jax.device_put(
        np.concatenate([np.asarray(in_maps[c][n]) for c in range(N_CORES)], axis=0),
        sharding) for n in in_names]
    cz = [jax.device_put(np.zeros((N_CORES * z.shape[0], *z.shape[1:]), z.dtype),
                         sharding) for z in zero_outs]
    outs = sharded(*ci, *cz)
    yi = out_names.index("y")
    return np.asarray(outs[yi]).reshape(N_CORES, *out_avals[yi].shape)


def run(x, w_qkv, b_qkv, w_proj, b_proj, trace=False):
    in_maps = _make_in_maps(np.asarray(x, dtype=np.float32),
                            np.asarray(w_qkv, dtype=np.float32),
                            np.asarray(b_qkv, dtype=np.float32),
                            np.asarray(w_proj, dtype=np.float32))
    y8 = _execute(in_maps)
    out = np.empty((B, T, C), dtype=np.float32)
    bp = np.asarray(b_proj, dtype=np.float32)
    for b in range(B):
        out[b] = y8[2 * b] + y8[2 * b + 1] + bp
    return out


def kernel(x, w_qkv, b_qkv, w_proj, b_proj):
    return run(x, w_qkv, b_qkv, w_proj, b_proj)


# revision 43
# speedup vs baseline: 1.0133x; 1.0133x over previous
"""Trainium2 Bass kernel for causal self-attention (muP scaling).

Full-input contract: kernel(**inputs) takes the complete tensors and returns
the complete [B, T, C] output. Internally the work is split over 8 NeuronCores
as (batch b = core//2) x (head-group g = core%2, 8 heads each):

  - each core computes q,k,v for its batch restricted to its 8 heads,
    runs causal attention for those heads, and multiplies by the matching
    512-row slice of w_proj, producing a partial [T, C] output.
  - the host sums the two partials per batch and adds b_proj. No on-device
    collectives are needed.

v2: single software-pipelined stream. The QKV projection is chunked per
query-block qb: chunk(qb) computes exactly the new k columns (tb=qb), the
q columns for qb, and v tiles 4qb..4qb+3 - the data attention block qb
needs - so the first exp issues ~25us into the kernel and the ScalarE exp
stream (the second-largest engine load, ~155us) overlaps the remaining
projection matmuls instead of waiting for a serial phase 1.

Attention runs per head PAIR (even head at qkT partitions 0:64, odd at
64:128); the two K=64 score matmuls write the two halves of one
[128, 1024] PSUM tile which a single ScalarE exp drains (2-segment
strided AP, muP 1/64 scale folded in; no max-subtraction - logits are
~N(0, 0.13)). Causal masking is a 0/1 triangular multiply on
diagonal-crossing tiles only. attT-out[d, tq] accumulates v_aug.T @ expT
with an appended ones column, so row 64 of the accumulator is the softmax
denominator for free. AV matmuls of block b are paced between the score
matmuls of block b+1 (and the projection chunk at qb boundaries), so the
PE never stalls on ScalarE.

Normalization v2: right after a block's AV matmuls the accumulator is
copied out unnormalized (bf16) and its denominator row appended to a
per-qb [8, 512] staging tile, freeing the PSUM bank immediately. One
reciprocal_approx_fast per qb (custom DVE op, ~5x faster than the 6.5
cyc/elem iterative InstReciprocal, batched over all 8 head-halves)
produces the scales, which GpSimd partition-broadcasts and one bf16
multiply applies in place. This removes the 106us of DVE InstReciprocal
the v1 kernel spent normalizing per block-half.

The output projection for qb is emitted interleaved with block qb+1's
attention; y rides DMA from SBUF after a DVE PSUM->SBUF cast-copy.
Activations ride bf16; measured end-to-end error vs the fp32 reference
is ~4e-3 relative.
"""

import sys

if "/opt/trn_rl_repo" not in sys.path:
    sys.path.insert(0, "/opt/trn_rl_repo")

import numpy as np
import ml_dtypes

import concourse.bass as bass
import concourse.mybir as mybir
import concourse.tile as tile
from concourse import bacc
from concourse.bass_utils import run_bass_kernel_spmd
from concourse.masks import make_upper_triangular

# Problem shape (hardcoded per contract).
B, T, C, H = 4, 2048, 1024, 16
HD = C // H            # 64
N_CORES = 8
HG = H // 2            # 8 heads per core
GC = HG * HD           # 512 columns of q/k/v per core
P = 128                # SBUF partitions
CT = C // P            # 8 contraction tiles over C
TT = T // P            # 16 time tiles of 128
QB = 4                 # tq blocks
QW = T // QB           # 512 wide
KT = T // P            # 16 tk tiles

_bf16np = ml_dtypes.bfloat16
F32 = mybir.dt.float32
BF16 = mybir.dt.bfloat16
FP8 = mybir.dt.float8e4
_fp8np = mybir.dt.np(FP8)

_COMPILED = None


def _build_nc(reps=1):
    import os
    opts = {
        "pipeline": os.environ.get("KV2_PIPELINE", "1") == "1",
        "fast_recip": os.environ.get("KV2_FASTRECIP", "1") == "1",
        "chunk_p1": os.environ.get("KV2_CHUNK_P1", "1") == "1",
        "debug_den": os.environ.get("KV2_DEBUG_DEN", "0") == "1",
    }
    nc = bacc.Bacc("TRN2", target_bir_lowering=False, debug=False,
                   num_devices=N_CORES)

    xT = nc.dram_tensor("xT", [C, T], BF16, kind="ExternalInput").ap()
    # fp8 copies of x / w_qk in DoubleRow-paired layout: row r = ctp*128+p,
    # free (i, t): value for contraction dim c = ctp*256 + i*128 + p.
    x8 = nc.dram_tensor("x8", [C // 2, 2 * T], FP8, kind="ExternalInput").ap()
    w8 = nc.dram_tensor("w8", [C // 2, 4 * GC], FP8, kind="ExternalInput").ap()
    w_v = nc.dram_tensor("w_v", [C, GC], BF16, kind="ExternalInput").ap()
    b_qk = nc.dram_tensor("b_qk", [2 * GC], F32, kind="ExternalInput").ap()
    b_v = nc.dram_tensor("b_v", [GC], F32, kind="ExternalInput").ap()
    w_pr = nc.dram_tensor("w_pr", [GC, C], BF16, kind="ExternalInput").ap()
    y = nc.dram_tensor("y", [T, C], F32, kind="ExternalOutput").ap()
    dden = (nc.dram_tensor("dden", [4 * HG, QW], F32, kind="ExternalOutput").ap()
            if opts["debug_den"] else None)
    opts = dict(opts, dden=dden)
    del opts["debug_den"]

    with tile.TileContext(nc) as tc:
        for _ in range(reps):
            _emit(nc, tc, xT, x8, w8, w_v, b_qk, b_v, w_pr, y, **opts)
    nc.finalize()
    return nc


def _emit(nc, tc, xT, x8, w8, w_v, b_qk, b_v, w_pr, y,
          pipeline=True, fast_recip=True, chunk_p1=True, dden=None):
    from contextlib import ExitStack

    ctx = ExitStack()
    with ctx:
        persist = ctx.enter_context(tc.tile_pool(name="persist", bufs=1))

        # ---- constants -------------------------------------------------
        tri = persist.tile([P, P], BF16, tag="tri")     # 0/1, 1 iff j >= i
        make_upper_triangular(nc, tri[:, :], val=1.0, diag=True)

        bqk_sb = persist.tile([P, CT], F32, tag="bqk")  # [128, 8] col jt
        nc.sync.dma_start(
            out=bqk_sb[:, :],
            in_=bass.AP(tensor=b_qk.tensor, offset=0, ap=[[1, P], [P, CT]]),
        )
        bv_sb = persist.tile([P, GC], F32, tag="bv")
        nc.gpsimd.dma_start(
            out=bv_sb[:, :],
            in_=bass.AP(tensor=b_v.tensor, offset=0, ap=[[0, P], [1, GC]]),
        )

        # ---- persistent tiles ------------------------------------------
        xts = [persist.tile([P, T], BF16, name=f"xT{ct}", tag=f"xT{ct}")
               for ct in range(CT)]
        x8t = [persist.tile([P, 2, T], FP8, name=f"x8_{cp}", tag=f"x8_{cp}")
               for cp in range(CT // 2)]
        w8t = [persist.tile([P, 2, 2 * GC], FP8, name=f"w8_{cp}", tag=f"w8_{cp}")
               for cp in range(CT // 2)]
        wvts = [persist.tile([P, GC], BF16, name=f"wv{ct}", tag=f"wv{ct}")
                for ct in range(CT)]
        qkT = [persist.tile([P, T], BF16, name=f"qkT{j}", tag=f"qkT{j}")
               for j in range(CT)]
        v_sb = [persist.tile([P, HG, HD + 1], BF16, name=f"v{t}", tag=f"v{t}")
                for t in range(TT)]
        att = [persist.tile([P, T], BF16, name=f"att{j}", tag=f"att{j}")
               for j in range(CT // 2)]
        wpr = [persist.tile([P, C], BF16, name=f"wpr{j}", tag=f"wpr{j}")
               for j in range(CT // 2)]

        # ---- input DMAs, deadline order, spread over the 3 hwdge queues
        _q = [nc.sync, nc.scalar, nc.gpsimd]
        qi = [0]

        def dma_in(out, in_):
            _q[qi[0] % 3].dma_start(out=out, in_=in_)
            qi[0] += 1

        # x8(tb=0) and the k half of w8 feed the very first matmuls.
        x8v = [x8[cp * P:(cp + 1) * P, :].rearrange("p (i t) -> p i t", i=2)
               for cp in range(CT // 2)]
        w8v = [w8[cp * P:(cp + 1) * P, :].rearrange("p (i j) -> p i j", i=2)
               for cp in range(CT // 2)]
        for cp in range(CT // 2):
            dma_in(x8t[cp][:, :, 0:QW], x8v[cp][:, :, 0:QW])
            dma_in(w8t[cp][:, :, GC:2 * GC], w8v[cp][:, :, GC:2 * GC])
        for cp in range(CT // 2):
            dma_in(w8t[cp][:, :, 0:GC], w8v[cp][:, :, 0:GC])
        for ct in range(CT):   # bf16 x cols for the first v tiles
            dma_in(xts[ct][:, 0:QW], xT[ct * P:(ct + 1) * P, 0:QW])
        for ct in range(CT):
            dma_in(wvts[ct][:, :], w_v[ct * P:(ct + 1) * P, :])
        for tb in range(1, QB):
            for cp in range(CT // 2):
                dma_in(x8t[cp][:, :, tb * QW:(tb + 1) * QW],
                       x8v[cp][:, :, tb * QW:(tb + 1) * QW])
            for ct in range(CT):
                dma_in(xts[ct][:, tb * QW:(tb + 1) * QW],
                       xT[ct * P:(ct + 1) * P, tb * QW:(tb + 1) * QW])
        for ct in range(CT // 2):
            dma_in(wpr[ct][:, :], w_pr[ct * P:(ct + 1) * P, :])

        # ---- pools -----------------------------------------------------
        expp = ctx.enter_context(tc.tile_pool(name="expp", bufs=20))
        nrm = ctx.enter_context(tc.tile_pool(name="nrm", bufs=2))
        yp = ctx.enter_context(tc.tile_pool(name="ysb", bufs=2))
        pss = ctx.enter_context(tc.tile_pool(name="pss", bufs=2, space="PSUM"))
        # role-split PSUM rings: block accumulators (long-lived, 2 slots is
        # exactly one block in flight) vs transient p1/proj tiles. Keeping
        # them separate preserves the ring-order invariants no matter how
        # many side jobs are interleaved between attention blocks.
        pacc = ctx.enter_context(tc.tile_pool(name="pacc", bufs=2, space="PSUM"))
        pfx = ctx.enter_context(tc.tile_pool(name="pfx", bufs=2, space="PSUM"))

        # ---- phase-1 chunk jobs (paired for PSUM bank alternation) -----
        def p1_kq(jts, tb):
            """k or q row tiles jts (absolute qkT index) for time block tb.

            fp8e4 DoubleRow: each matmul contracts 256 c-dims (2 planes of
            128) at half the per-column cost of bf16."""
            pss_ = [pfx.tile([P, QW], F32, name=f"p1_{jt}_{tb}", tag="fx")
                    for jt in jts]
            for cp in range(CT // 2):
                for i, jt in enumerate(jts):
                    nc.tensor.matmul(
                        pss_[i][:, :],
                        w8t[cp][:, :, jt * P:(jt + 1) * P],
                        x8t[cp][:, :, tb * QW:(tb + 1) * QW],
                        start=(cp == 0), stop=(cp == CT // 2 - 1),
                        perf_mode=mybir.MatmulPerfMode.DoubleRow,
                    )
            for i, jt in enumerate(jts):
                nc.vector.tensor_scalar_add(
                    out=qkT[jt][:, tb * QW:(tb + 1) * QW],
                    in0=pss_[i][:, :],
                    scalar1=bqk_sb[:, jt:jt + 1],
                )

        def p1_v(tts):
            pss_ = [pfx.tile([P, GC], F32, name=f"p1v_{tt}", tag="fx")
                    for tt in tts]
            for ct in range(CT):
                for i, tt in enumerate(tts):
                    nc.tensor.matmul(
                        pss_[i][:, :],
                        xts[ct][:, tt * P:(tt + 1) * P],
                        wvts[ct][:, :],
                        start=(ct == 0), stop=(ct == CT - 1),
                    )
            for i, tt in enumerate(tts):
                nc.vector.tensor_add(
                    out=v_sb[tt][:, :, 0:HD],
                    in0=pss_[i][:, :].rearrange("p (h e) -> p h e", e=HD),
                    in1=bv_sb[:, :].rearrange("p (h e) -> p h e", e=HD),
                )
                nc.vector.memset(v_sb[tt][:, :, HD:HD + 1], 1.0)

        # ---- attention emitters ---------------------------------------
        def emit_scores(hp, q0, kt, off, crossing):
            n = QW - off
            qT_t, kT_t = qkT[hp], qkT[CT // 2 + hp]
            ex = expp.tile([P, 2 * QW], BF16, tag="exp")
            ps = pss.tile([P, 2 * QW], F32, tag="sc")
            nc.tensor.matmul(
                ps[:, 0:n],
                kT_t[0:HD, kt * P:(kt + 1) * P],
                qT_t[0:HD, q0 + off:q0 + QW],
                start=True, stop=True,
            )
            nc.tensor.matmul(
                ps[:, QW:QW + n],
                kT_t[HD:P, kt * P:(kt + 1) * P],
                qT_t[HD:P, q0 + off:q0 + QW],
                start=True, stop=True,
            )
            # one exp over both heads: 2-segment strided view
            ps2 = ps[:, :].rearrange("p (s q) -> p s q", s=2)
            ex2 = ex[:, :].rearrange("p (s q) -> p s q", s=2)
            nc.scalar.activation(
                out=ex2[:, :, 0:n], in_=ps2[:, :, 0:n],
                func=mybir.ActivationFunctionType.Exp,
                scale=1.0 / HD,
            )
            if crossing:
                nc.vector.tensor_mul(
                    out=ex[:, 0:P], in0=ex[:, 0:P], in1=tri[:, :])
                nc.vector.tensor_mul(
                    out=ex[:, QW:QW + P], in0=ex[:, QW:QW + P], in1=tri[:, :])
            return ex

        def emit_av(st, i):
            (qb, hp, accs, exps, _prog) = st
            kt, off, n, ex = exps[i]
            last = i == len(exps) - 1
            nc.tensor.matmul(
                accs[0][0:HD + 1, off:QW],
                v_sb[kt][:, 2 * hp, :],
                ex[:, 0:n],
                start=(i == 0), stop=last,
                skip_group_check=True,
            )
            nc.tensor.matmul(
                accs[1][0:HD + 1, off:QW],
                v_sb[kt][:, 2 * hp + 1, :],
                ex[:, QW:QW + n],
                start=(i == 0), stop=last,
                skip_group_check=True,
            )

        def emit_post(st):
            """After a block's AVs: normalize straight out of PSUM.

            reciprocal_approx_fast is a single-pass custom DVE op (~0.6us on
            [1,512] vs 3.3us for the iterative InstReciprocal), so per-half
            normalization is cheap without any cross-partition batching."""
            (qb, hp, accs, exps, _prog) = st
            q0 = qb * QW
            for half, acc in ((0, accs[0]), (1, accs[1])):
                r0 = half * HD
                rec = nrm.tile([1, QW], F32, tag="rec", bufs=4)
                if fast_recip:
                    # custom DVE ops ignore the input AP's base partition
                    # (read physical row 0) - stage the denominator row at
                    # partition 0 with a plain copy (which does handle
                    # cross-base) before running the approx reciprocal.
                    den = nrm.tile([1, QW], F32, tag="den", bufs=2)
                    nc.vector.tensor_copy(out=den[0:1, :],
                                          in_=acc[HD:HD + 1, :])
                    if dden is not None:
                        nc.sync.dma_start(
                            out=dden[qb * CT + 2 * hp + half:
                                     qb * CT + 2 * hp + half + 1, :],
                            in_=den[0:1, :])
                    nc.vector.reciprocal_approx_fast(
                        out=rec[0:1, :], in_=den[0:1, :])
                else:
                    nc.vector.reciprocal(
                        out=rec[0:1, :], in_=acc[HD:HD + 1, :])
                bc = nrm.tile([HD, QW], F32, tag="bc", bufs=4)
                nc.gpsimd.partition_broadcast(
                    bc[0:HD, :], rec[0:1, :], channels=HD)
                nc.vector.tensor_mul(
                    out=att[hp][r0:r0 + HD, q0:q0 + QW],
                    in0=acc[0:HD, :],
                    in1=bc[0:HD, :],
                )

        yq = [0]

        def emit_proj_tile(tt):
            ysb = yp.tile([P, C], F32, tag="y")
            for nb in range(2):
                ps = pfx.tile([P, QW], F32, tag="fx", name=f"pj_{tt}_{nb}")
                for ct in range(CT // 2):
                    nc.tensor.matmul(
                        ps[:, :],
                        att[ct][:, tt * P:(tt + 1) * P],
                        wpr[ct][:, nb * QW:(nb + 1) * QW],
                        start=(ct == 0), stop=(ct == CT // 2 - 1),
                    )
                nc.vector.tensor_copy(
                    out=ysb[:, nb * QW:(nb + 1) * QW], in_=ps[:, :])
            (nc.sync if yq[0] % 2 == 0 else nc.gpsimd).dma_start(
                out=y[tt * P:(tt + 1) * P, :], in_=ysb[:, :])
            yq[0] += 1

        # ---- main pipelined loop --------------------------------------
        pend = [None]

        def pace_pend(frac_hi):
            st = pend[0]
            if st is None:
                return
            npend = len(st[3])
            hi = npend if frac_hi >= 1.0 else min(npend, int(frac_hi * npend))
            prog = st[4]
            for j in range(prog[0], hi):
                emit_av(st, j)
            prog[0] = max(prog[0], hi)
            if prog[0] >= npend:
                emit_post(st)
                pend[0] = None

        def p1_jobs_for(tb):
            return [
                lambda: p1_kq([4, 5], tb),
                lambda: p1_kq([0, 1], tb),
                lambda: p1_kq([6, 7], tb),
                lambda: p1_kq([2, 3], tb),
                lambda: p1_v([4 * tb, 4 * tb + 1]),
                lambda: p1_v([4 * tb + 2, 4 * tb + 3]),
            ]

        # prologue: the minimum block (qb0, hp0) needs - k heads 0/1 for
        # tb0 and q dims 0:256 for qb0. Everything else rides the side queue.
        if chunk_p1:
            for job in p1_jobs_for(0)[:2]:
                job()
        else:
            for tb in range(QB):
                for job in p1_jobs_for(tb):
                    job()

        for qb in range(QB):
            # side work interleaved into this qb's block gaps: the NEXT qb's
            # projection chunks (so its scores can start without a bunched
            # serial phase-1 stall) and the PREVIOUS qb's output projection.
            side = []
            if chunk_p1 and qb == 0:
                side += p1_jobs_for(0)[2:]
            if chunk_p1 and qb + 1 < QB:
                side += p1_jobs_for(qb + 1)
            if qb > 0:
                side += [(lambda tt=tt: emit_proj_tile(tt))
                         for tt in range(4 * (qb - 1), 4 * qb)]

            for hp in range(HG // 2):
                tiles = [(kt, 0, False) for kt in range(4 * qb)]
                tiles += [(4 * qb + a, P * a, True) for a in range(4)]
                q0 = qb * QW
                acc_e = pacc.tile([P, QW], F32, name=f"acc_e{hp}_{qb}", tag="acc")
                acc_o = pacc.tile([P, QW], F32, name=f"acc_o{hp}_{qb}", tag="acc")
                exps = []
                nt = len(tiles)
                for i, (kt, off, crossing) in enumerate(tiles):
                    ex = emit_scores(hp, q0, kt, off, crossing)
                    exps.append((kt, off, QW - off, ex))
                    pace_pend((i + 1) / nt)
                pace_pend(1.0)
                pend[0] = (qb, hp, (acc_e, acc_o), exps, [0])
                if not pipeline:
                    pace_pend(1.0)  # drain immediately (no AV/scores overlap)
                # spread side jobs over the 4 block gaps (ceil-even split)
                gaps_left = HG // 2 - hp
                take = (len(side) + gaps_left - 1) // gaps_left
                for _ in range(take):
                    side.pop(0)()
            assert not side

        # tail: last block, last projections
        pace_pend(1.0)
        for tt in range(4 * (QB - 1), 4 * QB):
            emit_proj_tile(tt)


def _get_compiled():
    global _COMPILED
    if _COMPILED is None:
        _COMPILED = _build_nc()
    return _COMPILED


def _pair_fp8(a):
    """[C, n] -> DoubleRow-paired fp8 [C//2, 2n]: row r=cp*128+p holds
    (plane i, col j) = a[cp*256 + i*128 + p, j]."""
    n = a.shape[1]
    return np.ascontiguousarray(
        a.reshape(CT // 2, 2, P, n).transpose(0, 2, 1, 3).reshape(C // 2, 2 * n)
    ).astype(_fp8np)


def _make_in_maps(x, w_qkv, b_qkv, w_proj):
    in_maps = []
    for c in range(N_CORES):
        b, g = c // 2, c % 2
        s = slice(g * GC, (g + 1) * GC)
        xTb = np.ascontiguousarray(x[b].T)
        w_qk = np.concatenate(
            [w_qkv[:, s], w_qkv[:, C + g * GC:C + (g + 1) * GC]], axis=1)
        in_maps.append({
            "xT": xTb.astype(_bf16np),
            "x8": _pair_fp8(xTb),
            "w8": _pair_fp8(w_qk),
            "w_v": np.ascontiguousarray(
                w_qkv[:, 2 * C + g * GC:2 * C + (g + 1) * GC]).astype(_bf16np),
            "b_qk": np.ascontiguousarray(
                np.concatenate([b_qkv[s], b_qkv[C + g * GC:C + (g + 1) * GC]])),
            "b_v": np.ascontiguousarray(b_qkv[2 * C + g * GC:2 * C + (g + 1) * GC]),
            "w_pr": np.ascontiguousarray(w_proj[g * GC:(g + 1) * GC, :]).astype(_bf16np),
        })
    return in_maps


_RUNNER = None


def _get_runner():
    """Compile once, cache the jitted shard_map executable across calls."""
    global _RUNNER
    if _RUNNER is not None:
        return _RUNNER
    import jax
    from jax.sharding import Mesh, PartitionSpec, NamedSharding
    from jax.experimental.shard_map import shard_map
    from concourse.bass2jax import (_bass_exec_p, install_neuronx_cc_hook,
                                    partition_id_tensor)

    nc = _get_compiled()
    install_neuronx_cc_hook()
    partition_name = nc.partition_id_tensor.name if nc.partition_id_tensor else None
    in_names, out_names, out_avals, zero_outs = [], [], [], []
    for alloc in nc.m.functions[0].allocations:
        if not isinstance(alloc, mybir.MemoryLocationSet):
            continue
        name = alloc.memorylocations[0].name
        if alloc.kind == "ExternalInput":
            if name != partition_name:
                in_names.append(name)
        elif alloc.kind == "ExternalOutput":
            out_names.append(name)
            out_avals.append(jax.core.ShapedArray(tuple(alloc.tensor_shape),
                                                  mybir.dt.np(alloc.dtype)))
            zero_outs.append(np.zeros(tuple(alloc.tensor_shape),
                                      mybir.dt.np(alloc.dtype)))
    all_in = list(in_names) + list(out_names)
    if partition_name:
        all_in.append(partition_name)

    def _body(*args):
        ops = list(args)
        if partition_name:
            ops.append(partition_id_tensor())
        return tuple(_bass_exec_p.bind(
            *ops, out_avals=tuple(out_avals), in_names=tuple(all_in),
            out_names=tuple(out_names), lowering_input_output_aliases=(),
            sim_require_finite=True, sim_require_nnan=True, nc=nc))

    devices = jax.devices()[:N_CORES]
    mesh = Mesh(np.asarray(devices), ("core",))
    sharded = jax.jit(shard_map(
        _body, mesh=mesh,
        in_specs=(PartitionSpec("core"),) * (len(in_names) + len(out_avals)),
        out_specs=(PartitionSpec("core"),) * len(out_avals), check_rep=False),
        keep_unused=True)
    sharding = NamedSharding(mesh, PartitionSpec("core"))
    _RUNNER = (sharded, in_names, zero_outs, sharding, out_avals, out_names)
    return _RUNNER


def _execute(in_maps):
    import jax
    sharded, in_names, zero_outs, sharding, out_avals, out_names = _get_runner()
    ci = [jax.device_put(
        np.concatenate([np.asarray(in_maps[c][n]) for c in range(N_CORES)], axis=0),
        sharding) for n in in_names]
    cz = [jax.device_put(np.zeros((N_CORES * z.shape[0], *z.shape[1:]), z.dtype),
                         sharding) for z in zero_outs]
    outs = sharded(*ci, *cz)
    yi = out_names.index("y")
    return np.asarray(outs[yi]).reshape(N_CORES, *out_avals[yi].shape)


def run(x, w_qkv, b_qkv, w_proj, b_proj, trace=False):
    in_maps = _make_in_maps(np.asarray(x, dtype=np.float32),
                            np.asarray(w_qkv, dtype=np.float32),
                            np.asarray(b_qkv, dtype=np.float32),
                            np.asarray(w_proj, dtype=np.float32))
    y8 = _execute(in_maps)
    out = np.empty((B, T, C), dtype=np.float32)
    bp = np.asarray(b_proj, dtype=np.float32)
    for b in range(B):
        out[b] = y8[2 * b] + y8[2 * b + 1] + bp
    return out


def kernel(x, w_qkv, b_qkv, w_proj, b_proj):
    return run(x, w_qkv, b_qkv, w_proj, b_proj)
